# revision 1
# baseline (speedup 1.0000x reference)
"""HSIC pairwise loss kernel for trn2 (8 NeuronCores).

Math: reference builds K_c = (w^2 w^2T) * (E_c E_c^T), M_c = R K_c, and sums
tr(M_i M_j) over i<j. With F_c = w^2 * E_c (row scaling), R the centering
matrix (idempotent):
    tr(R K_i R K_j) = ||G_i^T G_j||_F^2,  G_c = F_c - colmean(F_c)
and with A_ij = F_i^T F_j, s_c = F_c^T 1:
    G_i^T G_j = A_ij - (1/n) s_i s_j^T
so loss = sum_{i<j} ||A_ij - s_i s_j^T / n||_F^2 / (n-1)^2.

Device work: the 45 A_ij blocks [256,256] (contraction over n=4096) at
half-chunk granularity (20 units of 128 cols). Each core loads 9 units
(18.9 MB) and computes 8 matmul windows per k-tile into 8 PSUM banks,
fp32 data issued as float32r (full-rate on the PE at N>=256). A ones
column folded into the moving layout yields the column sums s for free.
Host assembles quadrants, applies the rank-1 centering correction and the
final scalar reduction in float64 (~3 MFLOP).
"""

import numpy as np
from contextlib import ExitStack

import concourse.bass as bass
import concourse.tile as tile
from concourse import bacc, mybir
from concourse import bass_utils

N = 4096
KT = 32            # k tiles of 128 rows
UNITS = 9          # half-chunk units per core
DCOLS = UNITS * 128          # 1152 data cols per k-tile
ROW = DCOLS + 2              # + two ones columns (fp32r needs even N)

# 8 cores x 9 units (of 20 half-chunks); covers all 180 cross-parent
# half-pairs via the fixed window pattern below (found by search).
ASSIGN = [
    [0, 19, 9, 4, 3, 18, 13, 17, 1],
    [10, 5, 8, 18, 4, 12, 9, 16, 15],
    [5, 17, 12, 9, 18, 2, 7, 14, 11],
    [14, 7, 12, 11, 2, 13, 1, 15, 16],
    [0, 17, 3, 19, 4, 6, 12, 11, 15],
    [14, 2, 9, 16, 5, 3, 19, 0, 6],
    [17, 18, 13, 11, 1, 6, 5, 8, 10],
    [3, 2, 19, 0, 14, 4, 7, 10, 8],
]

# (stat_slot, moving_start_col, n_cols). Moving cols 640..1151 are slots
# 5..8; col 1152 is the ones column (windows ending there also yield s).
WINDOWS = [
    (0, 640, 512),
    (1, 640, 512),
    (2, 640, 512),
    (3, 640, 512),
    (4, 640, 512),
    (5, 768, 386),
    (6, 896, 258),
    (7, 1024, 130),
]
OUT_COLS = sum(w[2] for w in WINDOWS)   # 3331

_CACHE = {}


def _build():
    f32 = mybir.dt.float32
    f32r = mybir.dt.float32r
    nc = bacc.Bacc("TRN2", target_bir_lowering=False, debug=False,
                   num_devices=8)
    x = nc.dram_tensor("x", [N, DCOLS], f32, kind="ExternalInput").ap()
    w = nc.dram_tensor("w", [128, KT], f32, kind="ExternalInput").ap()
    out = nc.dram_tensor("out", [128, OUT_COLS], f32,
                         kind="ExternalOutput").ap()

    with tile.TileContext(nc) as tc:
        with ExitStack() as ctx:
            spool = ctx.enter_context(tc.tile_pool(name="sw", bufs=1))
            xpool = ctx.enter_context(tc.tile_pool(name="xs", bufs=4))
            fpool = ctx.enter_context(tc.tile_pool(name="f", bufs=KT))
            psum = ctx.enter_context(tc.tile_pool(name="ps", bufs=1,
                                                  space="PSUM"))
            opool = ctx.enter_context(tc.tile_pool(name="o", bufs=1))

            swt = spool.tile([128, KT], f32, tag="swraw")
            nc.sync.dma_start(swt[:], w)
            sw2 = spool.tile([128, KT], f32, tag="swsq")
            nc.scalar.square(sw2[:], swt[:])
            one = spool.tile([128, 2], f32, tag="one")
            nc.vector.memset(one[:], 1.0)

            ps = []
            for i, (_, _, nw) in enumerate(WINDOWS):
                pst = psum.tile([128, nw], f32, tag=f"ps{i}", name=f"ps{i}")
                ps.append(pst)

            for k in range(KT):
                xr = xpool.tile([128, DCOLS], f32)
                nc.sync.dma_start(xr[:], x[k * 128:(k + 1) * 128, :])
                ft = fpool.tile([128, ROW], f32r)
                nc.vector.tensor_copy(ft[:, DCOLS:ROW], one[:])
                nc.scalar.mul(ft[:, 0:DCOLS], xr[:], sw2[:, k:k + 1])
                for wi, (s, mc, nw) in enumerate(WINDOWS):
                    nc.tensor.matmul(
                        ps[wi][:, 0:nw],
                        ft[:, s * 128:(s + 1) * 128],
                        ft[:, mc:mc + nw],
                        start=(k == 0),
                        stop=(k == KT - 1),
                    )

            ot = opool.tile([128, OUT_COLS], f32)
            col = 0
            for wi, (s, mc, nw) in enumerate(WINDOWS):
                eng = nc.vector if wi % 2 == 0 else nc.scalar
                if eng is nc.vector:
                    eng.tensor_copy(ot[:, col:col + nw], ps[wi][:, 0:nw])
                else:
                    eng.copy(ot[:, col:col + nw], ps[wi][:, 0:nw])
                col += nw
            nc.sync.dma_start(out, ot[:])
    nc.compile()
    return nc


def _get_nc():
    if "nc" not in _CACHE:
        _CACHE["nc"] = _build()
    return _CACHE["nc"]


def _in_maps(X, w):
    wt = np.ascontiguousarray(w.reshape(KT, 128).T)
    maps = []
    for units in ASSIGN:
        xc = np.concatenate([X[:, u * 128:(u + 1) * 128] for u in units],
                            axis=1)
        maps.append({"x": np.ascontiguousarray(xc), "w": wt})
    return maps


def _assemble(outs):
    quad = {}
    svec = {}
    for c, units in enumerate(ASSIGN):
        o = outs[c].astype(np.float64)
        col = 0
        for (s, mc, nw) in WINDOWS:
            su = units[s]
            block = o[:, col:col + nw]
            col += nw
            m0 = mc // 128
            for t in range((nw - (2 if nw % 128 else 0)) // 128):
                quad[(su, units[m0 + t])] = block[:, t * 128:(t + 1) * 128]
            if nw % 128:
                svec[su] = block[:, nw - 2]
    loss = 0.0
    for i in range(10):
        s_i = np.concatenate([svec[2 * i], svec[2 * i + 1]])
        for j in range(i + 1, 10):
            s_j = np.concatenate([svec[2 * j], svec[2 * j + 1]])
            A = np.empty((256, 256))
            for a in range(2):
                for b in range(2):
                    u, v = 2 * i + a, 2 * j + b
                    q = quad[(u, v)] if (u, v) in quad else quad[(v, u)].T
                    A[a * 128:(a + 1) * 128, b * 128:(b + 1) * 128] = q
            C = A - np.outer(s_i, s_j) / float(N)
            loss += float((C * C).sum())
    loss /= float((N - 1) * (N - 1))
    return np.asarray([loss], np.float32)


def kernel(final_readout, weight, _trace=False):
    X = np.ascontiguousarray(np.asarray(final_readout, np.float32))
    w = np.asarray(weight, np.float32)
    nc = _get_nc()
    res = bass_utils.run_bass_kernel_spmd(
        nc, _in_maps(X, w), core_ids=list(range(8)), trace=_trace)
    _CACHE["last_results"] = res
    return _assemble([r["out"] for r in res.results])



# revision 2
# speedup vs baseline: 2.0026x; 2.0026x over previous
"""HSIC pairwise loss kernel for trn2 (8 NeuronCores).

Math: reference builds K_c = (w^2 w^2T) * (E_c E_c^T), M_c = R K_c, and sums
tr(M_i M_j) over i<j. With F_c = w^2 * E_c (row scaling), R the centering
matrix (idempotent):
    tr(R K_i R K_j) = ||G_i^T G_j||_F^2,  G_c = F_c - colmean(F_c)
and with A_ij = F_i^T F_j, s_c = F_c^T 1, t_c = F_c s_c:
    ||G_i^T G_j||_F^2 = ||A_ij||_F^2 - (2/n) t_i.t_j + ||s_i||^2 ||s_j||^2 / n^2

Device work: the 45 A_ij blocks [256,256] (contraction over n=4096) at
half-chunk granularity (20 units of 128 cols). Each core loads 9 units,
pre-scaled by w^2 and cast to fp8e4m3 on host, laid out as 16 k-macrotiles
of 256 rows ([128 partitions, 2 k-subtiles, 1152 cols]). Matmuls run in
fp8 DoubleRow perf mode (256-deep contraction per instruction, double PE
rate), accumulating 8 PSUM windows over the 16 macrotiles. Output is the
raw A blocks cast to bf16. Host computes the rank-1 centering corrections
(s, t) exactly in float64 and the final scalar reduction.
"""

import numpy as np
import ml_dtypes
from contextlib import ExitStack

import concourse.bass as bass
import concourse.tile as tile
from concourse import bacc, mybir
from concourse import bass_utils

N = 4096
MT = 16            # k macrotiles of 256 rows (2 DoubleRow subtiles of 128)
UNITS = 9          # half-chunk units per core
DCOLS = UNITS * 128          # 1152 data cols

# 8 cores x 9 units (of 20 half-chunks); covers all 180 cross-parent
# half-pairs via the fixed window pattern below (found by search).
ASSIGN = [
    [0, 19, 9, 4, 3, 18, 13, 17, 1],
    [10, 5, 8, 18, 4, 12, 9, 16, 15],
    [5, 17, 12, 9, 18, 2, 7, 14, 11],
    [14, 7, 12, 11, 2, 13, 1, 15, 16],
    [0, 17, 3, 19, 4, 6, 12, 11, 15],
    [14, 2, 9, 16, 5, 3, 19, 0, 6],
    [17, 18, 13, 11, 1, 6, 5, 8, 10],
    [3, 2, 19, 0, 14, 4, 7, 10, 8],
]

# (stat_slot, moving_start_col, n_cols): bipartite {0-4}x{5-8} + clique {5-8}.
WINDOWS = [
    (0, 640, 512),
    (1, 640, 512),
    (2, 640, 512),
    (3, 640, 512),
    (4, 640, 512),
    (5, 768, 384),
    (6, 896, 256),
    (7, 1024, 128),
]
OUT_COLS = sum(w[2] for w in WINDOWS)   # 3328

_CACHE = {}


def _build():
    f32 = mybir.dt.float32
    bf16 = mybir.dt.bfloat16
    f8 = mybir.dt.float8e4
    DR = mybir.MatmulPerfMode.DoubleRow
    nc = bacc.Bacc("TRN2", target_bir_lowering=False, debug=False,
                   num_devices=8)
    x = nc.dram_tensor("x", [MT * 128, 2, DCOLS], f8,
                       kind="ExternalInput").ap()
    out = nc.dram_tensor("out", [128, OUT_COLS], bf16,
                         kind="ExternalOutput").ap()

    with tile.TileContext(nc) as tc:
        with ExitStack() as ctx:
            xpool = ctx.enter_context(tc.tile_pool(name="xs", bufs=6))
            psum = ctx.enter_context(tc.tile_pool(name="ps", bufs=1,
                                                  space="PSUM"))
            opool = ctx.enter_context(tc.tile_pool(name="o", bufs=1))

            ps = []
            for i, (_, _, nw) in enumerate(WINDOWS):
                pst = psum.tile([128, nw], f32, tag=f"ps{i}", name=f"ps{i}")
                ps.append(pst)

            for m in range(MT):
                xt = xpool.tile([128, 2, DCOLS], f8)
                nc.sync.dma_start(xt[:], x[m * 128:(m + 1) * 128, :, :])
                for wi, (s, mc, nw) in enumerate(WINDOWS):
                    nc.tensor.matmul(
                        ps[wi][:, 0:nw],
                        xt[:, :, s * 128:(s + 1) * 128],
                        xt[:, :, mc:mc + nw],
                        start=(m == 0),
                        stop=(m == MT - 1),
                        perf_mode=DR,
                    )

            ot = opool.tile([128, OUT_COLS], bf16)
            col = 0
            for wi, (s, mc, nw) in enumerate(WINDOWS):
                # split the PSUM->SBUF casts: big windows on the (faster)
                # vector engine, small tail windows on scalar
                if wi < 6:
                    nc.vector.tensor_copy(ot[:, col:col + nw], ps[wi][:, 0:nw])
                else:
                    nc.scalar.copy(ot[:, col:col + nw], ps[wi][:, 0:nw])
                nc.sync.dma_start(out[:, col:col + nw], ot[:, col:col + nw])
                col += nw
    nc.compile()
    return nc


def _get_nc():
    if "nc" not in _CACHE:
        _CACHE["nc"] = _build()
    return _CACHE["nc"]


def _in_maps(F32):
    maps = []
    for units in ASSIGN:
        xc = np.concatenate([F32[:, u * 128:(u + 1) * 128] for u in units],
                            axis=1)
        # [4096, 1152] -> [16 macrotiles, 128 partitions, 2 subtiles, 1152]
        xc = xc.reshape(MT, 2, 128, DCOLS).transpose(0, 2, 1, 3)
        x8 = np.ascontiguousarray(xc).astype(ml_dtypes.float8_e4m3)
        maps.append({"x": x8.reshape(MT * 128, 2, DCOLS)})
    return maps


def _assemble(outs, F64):
    quad = {}
    for c, units in enumerate(ASSIGN):
        o = outs[c].astype(np.float64)
        col = 0
        for (s, mc, nw) in WINDOWS:
            su = units[s]
            m0 = mc // 128
            for t in range(nw // 128):
                quad[(su, units[m0 + t])] = o[:, col + t * 128:
                                              col + (t + 1) * 128]
            col += nw
    # exact centering stats in f64
    s_vec = [F64[:, i * 256:(i + 1) * 256].sum(axis=0) for i in range(10)]
    t_vec = [F64[:, i * 256:(i + 1) * 256] @ s_vec[i] for i in range(10)]
    loss = 0.0
    for i in range(10):
        for j in range(i + 1, 10):
            asq = 0.0
            for a in range(2):
                for b in range(2):
                    u, v = 2 * i + a, 2 * j + b
                    q = quad[(u, v)] if (u, v) in quad else quad[(v, u)]
                    asq += float((q * q).sum())
            loss += (asq - (2.0 / N) * float(t_vec[i] @ t_vec[j])
                     + float(s_vec[i] @ s_vec[i]) * float(s_vec[j] @ s_vec[j])
                     / float(N * N))
    loss /= float((N - 1) * (N - 1))
    return np.asarray([loss], np.float32)


def kernel(final_readout, weight, _trace=False):
    X = np.asarray(final_readout, np.float32)
    w = np.asarray(weight, np.float32)
    F64 = (w.astype(np.float64) ** 2) * X.astype(np.float64)
    F32 = F64.astype(np.float32)
    nc = _get_nc()
    res = bass_utils.run_bass_kernel_spmd(
        nc, _in_maps(F32), core_ids=list(range(8)), trace=_trace)
    _CACHE["last_results"] = res
    return _assemble([r["out"] for r in res.results], F64)


# revision 4
# speedup vs baseline: 2.0277x; 1.0125x over previous
"""HSIC pairwise loss kernel for trn2 (8 NeuronCores).

Math: reference builds K_c = (w^2 w^2T) * (E_c E_c^T), M_c = R K_c, and sums
tr(M_i M_j) over i<j. With F_c = w^2 * E_c (row scaling), R the centering
matrix (idempotent):
    tr(R K_i R K_j) = ||G_i^T G_j||_F^2,  G_c = F_c - colmean(F_c)
and with A_ij = F_i^T F_j, s_c = F_c^T 1, t_c = F_c s_c:
    ||G_i^T G_j||_F^2 = ||A_ij||_F^2 - (2/n) t_i.t_j + ||s_i||^2 ||s_j||^2 / n^2

Device work: the 45 A_ij blocks [256,256] (contraction over n=4096) at
half-chunk granularity (20 units of 128 cols). Each core loads 9 units,
pre-scaled by w^2 and cast to fp8e4m3 on host, laid out as 16 k-macrotiles
of 256 rows ([128 partitions, 2 k-subtiles, 1152 cols]). Matmuls run in
fp8 DoubleRow perf mode (256-deep contraction per instruction, double PE
rate), accumulating the pair windows in PSUM over the 16 macrotiles. The
whole per-core input (4.7 MB) is SBUF-resident; loads are paired (4.6 KB
per partition line) and issued alternately from the Sync and Activation
sequencers to halve descriptor-generation latency. Output is the raw A
blocks cast to bf16, written by two DMAs (one per DGE sequencer). Host
computes the rank-1 centering corrections (s, t) exactly in float64 and
the final scalar reduction.
"""

import numpy as np
import ml_dtypes
from contextlib import ExitStack

import concourse.bass as bass
import concourse.tile as tile
from concourse import bacc, mybir
from concourse import bass_utils

N = 4096
MT = 16            # k macrotiles of 256 rows (2 DoubleRow subtiles of 128)
UNITS = 9          # half-chunk units per core
DCOLS = UNITS * 128          # 1152 data cols

# 8 cores x 9 units (of 20 half-chunks); covers all 180 cross-parent
# half-pairs via the fixed window pattern below (found by search).
ASSIGN = [
    [0, 19, 9, 4, 3, 18, 13, 17, 1],
    [10, 5, 8, 18, 4, 12, 9, 16, 15],
    [5, 17, 12, 9, 18, 2, 7, 14, 11],
    [14, 7, 12, 11, 2, 13, 1, 15, 16],
    [0, 17, 3, 19, 4, 6, 12, 11, 15],
    [14, 2, 9, 16, 5, 3, 19, 0, 6],
    [17, 18, 13, 11, 1, 6, 5, 8, 10],
    [3, 2, 19, 0, 14, 4, 7, 10, 8],
]

# (stat_slot, moving_start_col, n_cols): bipartite {0-4}x{5-8} + clique {5-8}.
WINDOWS = [
    (0, 640, 512),
    (1, 640, 512),
    (2, 640, 512),
    (3, 640, 512),
    (4, 640, 512),
    (5, 768, 384),
    (6, 896, 256),
    (7, 1024, 128),
]
OUT_COLS = sum(w[2] for w in WINDOWS)
# split casts/output DMAs: windows [0:SPLIT_W) -> sync DMA, rest -> scalar
SPLIT_W = 4
SPLIT_COL = sum(w[2] for w in WINDOWS[:SPLIT_W])

_CACHE = {}


def _build():
    f32 = mybir.dt.float32
    bf16 = mybir.dt.bfloat16
    f8 = mybir.dt.float8e4
    DR = mybir.MatmulPerfMode.DoubleRow
    nc = bacc.Bacc("TRN2", target_bir_lowering=False, debug=False,
                   num_devices=8)
    # row 128*q + p holds [pair member j, k-subtile s, col c] for F row
    # 512*q + 256*j + 128*s + p
    x = nc.dram_tensor("x", [(MT // 2) * 128, 2, 2, DCOLS], f8,
                       kind="ExternalInput").ap()
    out = nc.dram_tensor("out", [128, OUT_COLS], bf16,
                         kind="ExternalOutput").ap()

    with tile.TileContext(nc) as tc:
        with ExitStack() as ctx:
            xpool = ctx.enter_context(tc.tile_pool(name="xs", bufs=1))
            psum = ctx.enter_context(tc.tile_pool(name="ps", bufs=1,
                                                  space="PSUM"))
            opool = ctx.enter_context(tc.tile_pool(name="o", bufs=1))

            ps = []
            for i, (_, _, nw) in enumerate(WINDOWS):
                pst = psum.tile([128, nw], f32, tag=f"ps{i}", name=f"ps{i}")
                ps.append(pst)

            # pair 0 loads split into two half-tiles so macrotile 0's
            # matmuls start as soon as the first 295KB lands
            t0a = xpool.tile([128, 2, DCOLS], f8, name="t0a")
            t0b = xpool.tile([128, 2, DCOLS], f8, name="t0b")
            nc.sync.dma_start(t0a[:], x[0:128, 0:1, :, :])
            nc.scalar.dma_start(t0b[:], x[0:128, 1:2, :, :])
            pair_tiles = []
            for q in range(1, MT // 2):
                tq = xpool.tile([128, 2, 2, DCOLS], f8, name=f"t{q}")
                eng = nc.sync if q % 2 == 0 else nc.scalar
                eng.dma_start(tq[:], x[q * 128:(q + 1) * 128, :, :, :])
                pair_tiles.append(tq)

            def mt_view(m):
                if m == 0:
                    return t0a
                if m == 1:
                    return t0b
                return pair_tiles[m // 2 - 1][:, m % 2, :, :]

            for m in range(MT):
                xt = mt_view(m)
                for wi, (s, mc, nw) in enumerate(WINDOWS):
                    nc.tensor.matmul(
                        ps[wi][:, 0:nw],
                        xt[:, :, s * 128:(s + 1) * 128],
                        xt[:, :, mc:mc + nw],
                        start=(m == 0),
                        stop=(m == MT - 1),
                        perf_mode=DR,
                    )

            ot = opool.tile([128, OUT_COLS], bf16)
            col = 0
            for wi, (s, mc, nw) in enumerate(WINDOWS):
                # big windows cast on the (faster) vector engine; the two
                # small tail windows on gpsimd-free scalar
                if wi < 6:
                    nc.vector.tensor_copy(ot[:, col:col + nw], ps[wi][:, 0:nw])
                else:
                    nc.scalar.copy(ot[:, col:col + nw], ps[wi][:, 0:nw])
                col += nw
            nc.sync.dma_start(out[:, 0:SPLIT_COL], ot[:, 0:SPLIT_COL])
            nc.scalar.dma_start(out[:, SPLIT_COL:OUT_COLS],
                                ot[:, SPLIT_COL:OUT_COLS])
    nc.compile()
    return nc


def _get_nc():
    if "nc" not in _CACHE:
        _CACHE["nc"] = _build()
    return _CACHE["nc"]


def _in_maps(F32):
    maps = []
    for units in ASSIGN:
        xc = np.concatenate([F32[:, u * 128:(u + 1) * 128] for u in units],
                            axis=1)
        # [4096, 1152] -> [8 pairs, 128 partitions, 2 members, 2 subtiles, C]
        xc = xc.reshape(MT // 2, 2, 2, 128, DCOLS).transpose(0, 3, 1, 2, 4)
        x8 = np.ascontiguousarray(xc).astype(ml_dtypes.float8_e4m3)
        maps.append({"x": x8.reshape((MT // 2) * 128, 2, 2, DCOLS)})
    return maps


def _assemble(outs, F64):
    quad = {}
    for c, units in enumerate(ASSIGN):
        o = outs[c].astype(np.float64)
        col = 0
        for (s, mc, nw) in WINDOWS:
            su = units[s]
            m0 = mc // 128
            for t in range(nw // 128):
                quad[(su, units[m0 + t])] = o[:, col + t * 128:
                                              col + (t + 1) * 128]
            col += nw
    # exact centering stats in f64
    s_vec = [F64[:, i * 256:(i + 1) * 256].sum(axis=0) for i in range(10)]
    t_vec = [F64[:, i * 256:(i + 1) * 256] @ s_vec[i] for i in range(10)]
    loss = 0.0
    for i in range(10):
        for j in range(i + 1, 10):
            asq = 0.0
            for a in range(2):
                for b in range(2):
                    u, v = 2 * i + a, 2 * j + b
                    q = quad[(u, v)] if (u, v) in quad else quad[(v, u)]
                    asq += float((q * q).sum())
            loss += (asq - (2.0 / N) * float(t_vec[i] @ t_vec[j])
                     + float(s_vec[i] @ s_vec[i]) * float(s_vec[j] @ s_vec[j])
                     / float(N * N))
    loss /= float((N - 1) * (N - 1))
    return np.asarray([loss], np.float32)


def kernel(final_readout, weight, _trace=False):
    X = np.asarray(final_readout, np.float32)
    w = np.asarray(weight, np.float32)
    F64 = (w.astype(np.float64) ** 2) * X.astype(np.float64)
    F32 = F64.astype(np.float32)
    nc = _get_nc()
    res = bass_utils.run_bass_kernel_spmd(
        nc, _in_maps(F32), core_ids=list(range(8)), trace=_trace)
    _CACHE["last_results"] = res
    return _assemble([r["out"] for r in res.results], F64)
